# revision 25
# baseline (speedup 1.0000x reference)
"""Trainium2 Bass kernel for a 6-layer GPT forward pass (nn_GPT_21019569946962).

Sharding: sequence-parallel, 8 cores = 2 batches x 4 chunks of 256 tokens.
Per layer each core LayerNorms its 256 tokens, projects K (feature-major)
and V (token-major, via matmul with the activation as the stationary
operand — no PE transposes), and AllGathers K||V in one fused collective
within its 4-core replica group.  Attention runs with 128x128 PE-array
quadrant packing: head-pair scores are row-packed (two 64-deep
contractions concurrently), AV is col-packed (two 64-wide outputs
concurrently), softmax denominators accumulate via ones-matmuls into
packed PSUM rows.  Causal masking multiplies binary bf16 masks into the
exp'd scores on the DVE (all-SBUF 2-byte fast path).  The MLP and all
projections run at full 128x128 utilisation in bf16.  The LM head is
vocab-sharded within each 4-core group (12576 columns per core over its
batch's 1024 tokens); PSUM results DMA straight to DRAM.

Activations flow feature-major [D, tokens]; weights stream from HBM in
bf16; the residual stream and LN/softmax statistics stay fp32.
"""

import sys

sys.path.insert(0, "/opt/trn_rl_repo")

import numpy as np
import ml_dtypes

import concourse.bass as bass
import concourse.tile as tile
import concourse.mybir as mybir
from concourse import bacc
from concourse import bass_utils

BF16 = mybir.dt.bfloat16
F32 = mybir.dt.float32
AF = mybir.ActivationFunctionType
ALU = mybir.AluOpType

import os
SKIP_COLL = os.environ.get("SKIP_COLL", "0") == "1"

N_CORES = 8
NL = 6          # layers
D = 768
DT = 6          # d-tiles of 128
H = 12          # heads
HD = 64         # head dim
DFF = 3072
DFT = 24        # dff tiles of 128
VOC = 50304
VS = VOC // 4   # 12576 vocab shard per core (4-way within batch group)
B, L = 2, 1024
TOK = 256       # tokens per core
GTOK = 1024     # tokens per replica group (one batch)
EPS = 1e-6
NSC = 4 * DT + DFT + DT   # packed per-layer scales: ln1s,ln1b,ln2s,ln2b,w1b,w2b
VCH = 384       # lm-head vocab chunk
NVCH = 32       # full chunks; remainder 288
VREM = VS - NVCH * VCH


class GptKernel:
    def __init__(self, reps=1):
        self.reps = reps
        self.nc = self._build()

    # -------------------------------------------------------------- build
    def _build(self):
        nc = bacc.Bacc("TRN2", target_bir_lowering=False, debug=False,
                       enable_asserts=True, num_devices=N_CORES)
        self.nc = nc

        def din(name, shape, dt):
            return nc.dram_tensor(name, shape, dt, kind="ExternalInput").ap()

        self.x0 = din("x0", [D, TOK], F32)
        self.wq = din("wq", [NL, D, D], BF16)
        self.wk = din("wk", [NL, D, D], BF16)
        self.wv = din("wv", [NL, D, D], BF16)
        self.wo = din("wo", [NL, D, D], BF16)
        self.w1 = din("w1", [NL, D, DFF], BF16)
        self.w2 = din("w2", [NL, DFF, D], BF16)
        self.scal = din("scal", [NL, 128, NSC], F32)
        self.lnfs = din("lnfs", [D], F32)
        self.lnfb = din("lnfb", [D], F32)
        self.headw = din("headw", [D, VS], BF16)
        self.amask = din("amask", [8, 128, 2 * TOK], BF16)
        self.selc = din("selc", [2, 128, 128], BF16)
        self.out = nc.dram_tensor("out", [GTOK, VS], F32,
                                  kind="ExternalOutput").ap()

        with tile.TileContext(nc) as tc:
            self.tc = tc
            with (
                tc.tile_pool(name="const", bufs=1) as cp,
                tc.tile_pool(name="persist", bufs=1) as pp,
                tc.tile_pool(name="psum", bufs=1, space="PSUM") as psum,
                tc.tile_pool(name="dram", bufs=1, space="DRAM") as dram,
                tc.tile_pool(name="work", bufs=1) as wp,
            ):
                self.psum, self.dram, self.wp = psum, dram, wp
                self.ones_r = cp.tile([1, 128], F32)
                nc.vector.memset(self.ones_r[:], 1.0)
                self.ones_c = cp.tile([128, 1], BF16)
                nc.vector.memset(self.ones_c[:], 1.0)
                self.sel = cp.tile([128, 2, 128], BF16)
                nc.sync.dma_start(self.sel[:],
                                  self.selc.rearrange("s p q -> p s q"))
                self.mask_sb = pp.tile([128, 8, 2 * TOK], BF16)
                nc.sync.dma_start(self.mask_sb[:],
                                  self.amask.rearrange("k p t -> p k t"))
                self.xres = pp.tile([128, DT, TOK], F32)

                for rep in range(self.reps):
                    nc.sync.dma_start(
                        self.xres[:],
                        self.x0.rearrange("(dt p) t -> p dt t", p=128))
                    for l in range(NL):
                        self._layer(l, rep)
                    self._lm_head(rep)
        nc.compile()
        return nc

    # ------------------------------------------------------------ layernorm
    def _layernorm(self, xres, g, b, name):
        """xres [128, DT, TOK] f32 -> ln [128, DT, TOK] bf16."""
        nc, wp, psum = self.nc, self.wp, self.psum
        stat = psum.tile([128, TOK], F32, tag="mm", bufs=2, name=f"st_{name}")
        xbs = []
        for k in range(DT):
            xb = wp.tile([128, TOK], BF16, tag="xb", bufs=2, name=f"xb{k}_{name}")
            nc.vector.tensor_copy(xb[:], xres[:, k, :])
            xbs.append(xb)
        xqs = []
        for k in range(DT):
            xq = wp.tile([128, TOK], BF16, tag="xq", bufs=2, name=f"xq{k}_{name}")
            nc.scalar.activation(xq[:], xres[:, k, :], AF.Square,
                                 bias=0.0, scale=1.0)
            xqs.append(xq)
        for k in range(DT):
            nc.tensor.matmul(stat[0:1, :], self.ones_c[:], xbs[k][:],
                             start=(k == 0), stop=(k == DT - 1),
                             tile_position=(0, 0), skip_group_check=True)
        for k in range(DT):
            nc.tensor.matmul(stat[32:33, :], self.ones_c[:], xqs[k][:],
                             start=(k == 0), stop=(k == DT - 1),
                             tile_position=(0, 32), skip_group_check=True)
        mu = wp.tile([1, TOK], F32, tag="lnsc", bufs=8, name=f"mu_{name}")
        nc.vector.tensor_scalar_mul(mu[:], stat[0:1, :], 1.0 / D)
        msq = wp.tile([1, TOK], F32, tag="lnsc", bufs=8, name=f"msq_{name}")
        nc.vector.tensor_scalar_mul(msq[:], stat[32:33, :], 1.0 / D)
        mu2 = wp.tile([1, TOK], F32, tag="lnsc", bufs=8, name=f"mu2_{name}")
        nc.vector.tensor_mul(mu2[:], mu[:], mu[:])
        var = wp.tile([1, TOK], F32, tag="lnsc", bufs=8, name=f"va_{name}")
        nc.vector.tensor_sub(var[:], msq[:], mu2[:])
        vare = wp.tile([1, TOK], F32, tag="lnsc", bufs=8, name=f"ve_{name}")
        nc.vector.tensor_scalar_add(vare[:], var[:], EPS)
        sd = wp.tile([1, TOK], F32, tag="lnsc", bufs=8, name=f"sd_{name}")
        nc.scalar.activation(sd[:], vare[:], AF.Sqrt, bias=0.0, scale=1.0)
        rstd = wp.tile([1, TOK], F32, tag="lnsc", bufs=8, name=f"rstd_{name}")
        nc.vector.reciprocal(rstd[:], sd[:])
        nmr = wp.tile([1, TOK], F32, tag="lnsc", bufs=8, name=f"nmr_{name}")
        nc.vector.tensor_mul(nmr[:], mu[:], rstd[:])

        # bc[:, 0:TOK] = rstd broadcast, bc[:, TOK:2T] = mu*rstd broadcast
        bc = psum.tile([128, 2 * TOK], F32, tag="s", bufs=2, name=f"bc_{name}")
        nc.tensor.matmul(bc[:, 0:TOK], self.ones_r[:], rstd[:],
                         start=True, stop=True, skip_group_check=True)
        nc.tensor.matmul(bc[:, TOK:2 * TOK], self.ones_r[:], nmr[:],
                         start=True, stop=True, skip_group_check=True)

        ln = wp.tile([128, DT, TOK], BF16, tag=f"ln_{name[:3]}", bufs=1,
                     name=f"ln_{name}")
        for k in range(DT):
            u = wp.tile([128, TOK], F32, tag="lnu", bufs=2, name=f"u{k}_{name}")
            nc.vector.tensor_mul(u[:], xres[:, k, :], bc[:, 0:TOK])
            v = wp.tile([128, TOK], F32, tag="lnv", bufs=2, name=f"v{k}_{name}")
            nc.vector.tensor_sub(v[:], u[:], bc[:, TOK:2 * TOK])
            nc.scalar.activation(ln[:, k, :], v[:], AF.Identity,
                                 bias=b[:, k:k + 1], scale=g[:, k:k + 1])
        return ln

    # ------------------------------------------------------------ layer
    def _layer(self, l, rep):
        nc, wp, psum, dram = self.nc, self.wp, self.psum, self.dram
        nm = f"r{rep}l{l}"

        sc = wp.tile([128, NSC], F32, tag="sc", bufs=2, name=f"sc_{nm}")
        nc.sync.dma_start(sc[:], self.scal[l])
        g1, b1 = sc[:, 0:DT], sc[:, DT:2 * DT]
        g2, b2 = sc[:, 2 * DT:3 * DT], sc[:, 3 * DT:4 * DT]
        w1b = sc[:, 4 * DT:4 * DT + DFT]
        w2b = sc[:, 4 * DT + DFT:NSC]

        wq_sb = wp.tile([128, DT, D], BF16, tag="wq", bufs=1, name=f"wq_{nm}")
        nc.sync.dma_start(wq_sb[:],
                          self.wq[l].rearrange("(t p) d -> p t d", p=128))
        wk_sb = wp.tile([128, DT, D], BF16, tag="wk", bufs=1, name=f"wk_{nm}")
        nc.sync.dma_start(wk_sb[:],
                          self.wk[l].rearrange("(t p) d -> p t d", p=128))
        wv_sb = wp.tile([128, DT, D], BF16, tag="wv", bufs=1, name=f"wv_{nm}")
        nc.sync.dma_start(wv_sb[:],
                          self.wv[l].rearrange("(t p) d -> p t d", p=128))
        wo_sb = wp.tile([128, DT, D], BF16, tag="wo", bufs=1, name=f"wo_{nm}")
        nc.sync.dma_start(wo_sb[:],
                          self.wo[l].rearrange("(t p) d -> p t d", p=128))

        ln1 = self._layernorm(self.xres, g1, b1, f"ln1_{nm}")

        # ---- K projection (feature-major) + V projection (token-major)
        # into one staging tile, then a single fused AllGather.
        kst = wp.tile([128, DT * TOK], BF16, tag="kvst", bufs=1,
                      name=f"kst_{nm}")
        for m in range(DT):
            ps = psum.tile([128, TOK], F32, tag="mm", bufs=2,
                           name=f"pk{m}_{nm}")
            for kk in range(DT):
                nc.tensor.matmul(ps[:], wk_sb[:, kk, m * 128:(m + 1) * 128],
                                 ln1[:, kk, :],
                                 start=(kk == 0), stop=(kk == DT - 1))
            nc.vector.tensor_copy(kst[:, m * TOK:(m + 1) * TOK], ps[:])
        kin = dram.tile([128, DT * TOK], BF16, tag="kin", bufs=2,
                        name=f"kin_{nm}")
        nc.sync.dma_start(kin[:], kst[:])
        kout = dram.tile([4, 128, DT * TOK], BF16, tag="kout",
                         bufs=2, name=f"kout_{nm}")
        if not SKIP_COLL:
            nc.gpsimd.collective_compute(
                "AllGather", ALU.bypass, ins=[kin.opt()], outs=[kout.opt()],
                replica_groups=[[0, 1, 2, 3], [4, 5, 6, 7]])

        vst = wp.tile([128, 2 * D], BF16, tag="vst", bufs=1, name=f"vst_{nm}")
        for tb in range(2):
            for vh in range(2):
                ps = psum.tile([128, D // 2], F32, tag="mm", bufs=2,
                               name=f"pv{tb}_{vh}_{nm}")
                for kk in range(DT):
                    nc.tensor.matmul(ps[:],
                                     ln1[:, kk, tb * 128:(tb + 1) * 128],
                                     wv_sb[:, kk, vh * 384:(vh + 1) * 384],
                                     start=(kk == 0), stop=(kk == DT - 1),
                                     skip_group_check=True)
                off = tb * D + vh * 384
                nc.vector.tensor_copy(vst[:, off:off + 384], ps[:])
        vin = dram.tile([128, 2 * D], BF16, tag="vin", bufs=2,
                        name=f"vin_{nm}")
        nc.sync.dma_start(vin[:], vst[:])
        vout = dram.tile([4, 128, 2 * D], BF16, tag="vout",
                         bufs=2, name=f"vout_{nm}")
        if not SKIP_COLL:
            nc.gpsimd.collective_compute(
                "AllGather", ALU.bypass, ins=[vin.opt()], outs=[vout.opt()],
                replica_groups=[[0, 1, 2, 3], [4, 5, 6, 7]])

        # ---- Q projection (overlaps the gather)
        q_sb = wp.tile([128, DT, TOK], BF16, tag="q", bufs=1, name=f"q_{nm}")
        for m in range(DT):
            ps = psum.tile([128, TOK], F32, tag="mm", bufs=2,
                           name=f"pq{m}_{nm}")
            for kk in range(DT):
                nc.tensor.matmul(ps[:], wq_sb[:, kk, m * 128:(m + 1) * 128],
                                 ln1[:, kk, :],
                                 start=(kk == 0), stop=(kk == DT - 1))
            nc.vector.tensor_copy(q_sb[:, m, :], ps[:])
        q64 = wp.tile([64, 2, DT, TOK], BF16, tag="q64", bufs=1,
                      name=f"q64_{nm}")
        for h2 in range(2):
            nc.sync.dma_start(q64[:, h2], q_sb[64 * h2:64 * h2 + 64])

        # ---- load gathered K (feature-major) and V^T (token-major)
        kg64 = wp.tile([64, 2, DT, 4, TOK], BF16, tag="kg", bufs=1,
                       name=f"kg_{nm}")
        for r in range(4):
            for h2 in range(2):
                nc.sync.dma_start(
                    kg64[:, h2, :, r, :],
                    kout[r, 64 * h2:64 * h2 + 64, :].rearrange(
                        "p (dt t) -> p dt t", dt=DT))
        vt4 = wp.tile([128, 4, 2, D], BF16, tag="vt", bufs=1, name=f"vt_{nm}")
        for r in range(4):
            nc.sync.dma_start(
                vt4[:, r],
                vout[r].rearrange("p (tb d) -> p tb d", tb=2))
        vt = vt4[:].rearrange("p r tb d -> p (r tb) d")

        # ---- attention: head pairs, quadrant-packed
        at = wp.tile([128, DT, TOK], BF16, tag="at", bufs=1, name=f"at_{nm}")
        for j in range(DT):
            dn = psum.tile([128, TOK], F32, tag="dn", bufs=2,
                           name=f"dn{j}_{nm}")
            p_list = []
            for kb in range(8):
                s = psum.tile([128, 2 * TOK], F32, tag="s", bufs=2,
                              name=f"s{j}_{kb}_{nm}")
                r, tb = kb // 2, kb % 2
                nc.tensor.matmul(
                    s[:, 0:TOK], kg64[:, 0, j, r, tb * 128:(tb + 1) * 128],
                    q64[:, 0, j, :], start=True, stop=True)
                nc.tensor.matmul(
                    s[:, TOK:2 * TOK], kg64[:, 1, j, r, tb * 128:(tb + 1) * 128],
                    q64[:, 1, j, :], start=True, stop=True)
                pm = wp.tile([128, 2 * TOK], BF16, tag="pm", bufs=2,
                             name=f"pm{j}_{kb}_{nm}")
                nc.scalar.activation(pm[:], s[:], AF.Exp, bias=0.0, scale=0.125)
                p = wp.tile([128, 2 * TOK], BF16, tag="p", bufs=8,
                            name=f"p{j}_{kb}_{nm}")
                nc.vector.tensor_mul(p[:], pm[:], self.mask_sb[:, kb, :])
                p_list.append(p)
                nc.tensor.matmul(dn[0:1, :], self.ones_c[:],
                                 p[:, 0:TOK], start=(kb == 0), stop=(kb == 7),
                                 tile_position=(0, 0), skip_group_check=True)
                nc.tensor.matmul(dn[64:65, :], self.ones_c[:],
                                 p[:, TOK:2 * TOK], start=(kb == 0), stop=(kb == 7),
                                 tile_position=(0, 64), skip_group_check=True)
            ao = psum.tile([128, TOK], F32, tag="ao", bufs=2, name=f"ao{j}_{nm}")
            for kb in range(8):
                p = p_list[kb]
                nc.tensor.matmul(ao[0:64, :], vt[:, kb, 2 * j * 64:2 * j * 64 + 64],
                                 p[:, 0:TOK], start=(kb == 0), stop=(kb == 7),
                                 skip_group_check=True)
                nc.tensor.matmul(ao[64:128, :],
                                 vt[:, kb, (2 * j + 1) * 64:(2 * j + 2) * 64],
                                 p[:, TOK:2 * TOK], start=(kb == 0), stop=(kb == 7),
                                 skip_group_check=True)
            rd = wp.tile([128, TOK], BF16, tag="rd", bufs=2,
                         name=f"rd{j}_{nm}")
            nc.vector.memset(rd[:], 0.0)
            with nc.allow_low_precision(reason="softmax denom bcast in bf16"):
                nc.vector.reciprocal(rd[0:1, :], dn[0:1, :])
                nc.vector.reciprocal(rd[64:65, :], dn[64:65, :])
            bc = psum.tile([128, TOK], F32, tag="s", bufs=2,
                           name=f"bc{j}_{nm}")
            nc.tensor.matmul(bc[:], self.sel[:, 0, :], rd[:],
                             start=True, stop=True, skip_group_check=True)
            aosb = wp.tile([128, TOK], BF16, tag="aosb", bufs=2,
                           name=f"aosb{j}_{nm}")
            nc.vector.tensor_copy(aosb[:], ao[:])
            nc.vector.tensor_mul(at[:, j, :], aosb[:], bc[:])

        # ---- WO + residual
        for m in range(DT):
            ps = psum.tile([128, TOK], F32, tag="mm", bufs=2,
                           name=f"pwo{m}_{nm}")
            for j in range(DT):
                nc.tensor.matmul(ps[:], wo_sb[:, j, m * 128:(m + 1) * 128],
                                 at[:, j, :], start=(j == 0), stop=(j == DT - 1))
            nc.vector.tensor_add(self.xres[:, m, :], self.xres[:, m, :], ps[:])

        # ---- LN2 + MLP
        ln2 = self._layernorm(self.xres, g2, b2, f"ln2_{nm}")
        h1 = wp.tile([128, DFT, TOK], BF16, tag="h1", bufs=1, name=f"h1_{nm}")
        for blk in range(4):
            w1_sb = wp.tile([128, DT, DFF // 4], BF16, tag="w1", bufs=2,
                            name=f"w1_{blk}_{nm}")
            nc.sync.dma_start(
                w1_sb[:],
                self.w1[l, :, blk * 768:(blk + 1) * 768].rearrange(
                    "(t p) d -> p t d", p=128))
            for mi in range(6):
                m = blk * 6 + mi
                ps = psum.tile([128, TOK], F32, tag="mm", bufs=2,
                               name=f"ph1_{m}_{nm}")
                for kk in range(DT):
                    nc.tensor.matmul(ps[:],
                                     w1_sb[:, kk, mi * 128:(mi + 1) * 128],
                                     ln2[:, kk, :],
                                     start=(kk == 0), stop=(kk == DT - 1))
                nc.scalar.activation(h1[:, m, :], ps[:], AF.Gelu_apprx_tanh,
                                     bias=w1b[:, m:m + 1], scale=1.0)
        for blk in range(3):
            w2_sb = wp.tile([128, DFT, 2 * 128], BF16, tag="w2", bufs=2,
                            name=f"w2_{blk}_{nm}")
            nc.sync.dma_start(
                w2_sb[:],
                self.w2[l, :, blk * 256:(blk + 1) * 256].rearrange(
                    "(t p) d -> p t d", p=128))
            for mi in range(2):
                m = blk * 2 + mi
                ps = psum.tile([128, TOK], F32, tag="mm", bufs=2,
                               name=f"pw2_{m}_{nm}")
                for kk in range(DFT):
                    nc.tensor.matmul(ps[:],
                                     w2_sb[:, kk, mi * 128:(mi + 1) * 128],
                                     h1[:, kk, :],
                                     start=(kk == 0), stop=(kk == DFT - 1))
                mo = wp.tile([128, TOK], F32, tag="mo", bufs=2, name=f"mo{m}_{nm}")
                nc.scalar.activation(mo[:], ps[:], AF.Identity,
                                     bias=w2b[:, m:m + 1], scale=1.0)
                nc.vector.tensor_add(self.xres[:, m, :], self.xres[:, m, :], mo[:])

    # ------------------------------------------------------------ lm head
    def _lm_head(self, rep):
        nc, wp, psum, dram = self.nc, self.wp, self.psum, self.dram
        nm = f"r{rep}f"
        gf = wp.tile([128, DT], F32, tag="sc", bufs=2, name=f"gf_{nm}")
        nc.sync.dma_start(gf[:], self.lnfs.rearrange("(t p) -> p t", p=128))
        bf = wp.tile([128, DT], F32, tag="sc", bufs=2, name=f"bf_{nm}")
        nc.sync.dma_start(bf[:], self.lnfb.rearrange("(t p) -> p t", p=128))
        lnf = self._layernorm(self.xres, gf, bf, f"lnf_{nm}")

        fin = dram.tile([128, DT * TOK], BF16, tag="kin", bufs=2,
                        name=f"fin_{nm}")
        nc.sync.dma_start(fin[:], lnf[:].rearrange("p t d -> p (t d)"))
        fout = dram.tile([4, 128, DT * TOK], BF16, tag="kout", bufs=2,
                         name=f"fout_{nm}")
        if not SKIP_COLL:
            nc.gpsimd.collective_compute(
                "AllGather", ALU.bypass, ins=[fin.opt()], outs=[fout.opt()],
                replica_groups=[[0, 1, 2, 3], [4, 5, 6, 7]])
        fg4 = wp.tile([128, DT, 4, TOK], BF16, tag="kg", bufs=1, name=f"fg_{nm}")
        for r in range(4):
            nc.sync.dma_start(
                fg4[:, :, r, :],
                fout[r].rearrange("p (dt t) -> p dt t", dt=DT))
        fg = fg4[:].rearrange("p dt r t -> p dt (r t)")

        chunks = [(c * VCH, VCH) for c in range(NVCH)] + [(NVCH * VCH, VREM)]
        for c0, cn in chunks:
            hw = wp.tile([128, DT, VCH], BF16, tag="hw", bufs=2,
                         name=f"hw{c0}_{nm}")
            nc.sync.dma_start(
                hw[:, :, 0:cn],
                self.headw[:, c0:c0 + cn].rearrange("(t p) v -> p t v", p=128))
            for tb in range(8):
                ot = wp.tile([128, VCH], F32, tag="ot", bufs=1,
                             name=f"ot{c0}_{tb}_{nm}")
                for si, s0 in enumerate(range(0, cn, 512)):
                    sn = min(512, cn - s0)
                    ps = psum.tile([128, 512], F32, tag="s", bufs=2,
                                   name=f"hp{c0}_{tb}_{si}_{nm}")
                    for kk in range(DT):
                        nc.tensor.matmul(
                            ps[:, 0:sn],
                            fg[:, kk, tb * 128:(tb + 1) * 128],
                            hw[:, kk, s0:s0 + sn],
                            start=(kk == 0), stop=(kk == DT - 1),
                            skip_group_check=True)
                    if si % 2 == 0:
                        nc.vector.tensor_copy(ot[:, s0:s0 + sn], ps[:, 0:sn])
                    else:
                        nc.scalar.activation(ot[:, s0:s0 + sn], ps[:, 0:sn],
                                             AF.Identity, bias=0.0, scale=1.0)
                nc.sync.dma_start(
                    self.out[tb * 128:(tb + 1) * 128, c0:c0 + cn],
                    ot[:, 0:cn])


# ------------------------------------------------------------------ host side

_CACHE = {}


def _prep_inputs(inputs):
    ids = np.asarray(inputs["input_ids"])
    tok_emb = np.asarray(inputs["tok_emb"], dtype=np.float32)
    pos_emb = np.asarray(inputs["pos_emb"], dtype=np.float32)
    x = tok_emb[ids] + pos_emb[:L][None]          # [2, 1024, 768] f32

    bf = lambda a: np.ascontiguousarray(np.asarray(a, np.float32)).astype(ml_dtypes.bfloat16)
    f32 = lambda a: np.ascontiguousarray(np.asarray(a, np.float32))

    # packed per-layer scales: [NL, 128, NSC]; column k of row p is element
    # (k*128+p) of the flat [768] / [3072] vectors (partition-major tiles)
    scal = np.zeros((NL, 128, NSC), np.float32)
    def pack(dst_off, src, width):
        # src [NL, width*128] -> scal[:, :, dst_off:dst_off+width]
        scal[:, :, dst_off:dst_off + width] = src.reshape(NL, width, 128).transpose(0, 2, 1)
    pack(0, f32(inputs["ln1_s"]), DT)
    pack(DT, f32(inputs["ln1_b"]), DT)
    pack(2 * DT, f32(inputs["ln2_s"]), DT)
    pack(3 * DT, f32(inputs["ln2_b"]), DT)
    pack(4 * DT, f32(inputs["w1_b"]), DFT)
    pack(4 * DT + DFT, f32(inputs["w2_b"]), DT)

    shared = {
        "wq": bf(inputs["wq"]), "wk": bf(inputs["wk"]),
        "wv": bf(inputs["wv"]), "wo": bf(inputs["wo"]),
        "w1": bf(inputs["w1_k"]), "w2": bf(inputs["w2_k"]),
        "scal": scal,
        "lnfs": f32(inputs["lnf_s"]), "lnfb": f32(inputs["lnf_b"]),
    }
    head_bf = bf(inputs["head"])

    # selector constants for denominator broadcast: pattern s (0: rows 0/32,
    # 1: rows 64/96): sel[s][k, p] = 1 iff k == 64*s + 32*(p >= 64)
    selc = np.zeros((2, 128, 128), ml_dtypes.bfloat16)
    selc[0, 0, 0:64] = 1.0
    selc[0, 64, 64:128] = 1.0

    in_maps = []
    for c in range(N_CORES):
        g, j = c // 4, c % 4
        m = dict(shared)
        m["x0"] = np.ascontiguousarray(x[g, TOK * j:TOK * (j + 1)].T)
        m["headw"] = np.ascontiguousarray(head_bf[:, j * VS:(j + 1) * VS])
        m["selc"] = selc
        # binary causal mask, duplicated for the head pair: [8, 128, 512]
        am = np.zeros((8, 128, 2 * TOK), ml_dtypes.bfloat16)
        for kb in range(8):
            kgl = 128 * kb + np.arange(128)[:, None]
            qgl = TOK * j + np.arange(TOK)[None, :]
            vis = (kgl <= qgl).astype(ml_dtypes.bfloat16)
            am[kb, :, 0:TOK] = vis
            am[kb, :, TOK:2 * TOK] = vis
        m["amask"] = am
        in_maps.append(m)
    return in_maps


def _assemble(results):
    final = np.empty((B, L, VOC), np.float32)
    for c in range(N_CORES):
        g, j = c // 4, c % 4
        final[g, :, j * VS:(j + 1) * VS] = results[c]["out"]
    return final


def kernel(**inputs):
    if "k" not in _CACHE:
        _CACHE["k"] = GptKernel(reps=1)
    gk = _CACHE["k"]
    in_maps = _prep_inputs(inputs)
    res = bass_utils.run_bass_kernel_spmd(
        gk.nc, in_maps, core_ids=list(range(N_CORES)))
    _CACHE["last_results"] = res
    return _assemble(res.results)


# revision 26
# speedup vs baseline: 1.0406x; 1.0406x over previous
"""Trainium2 Bass kernel for a 6-layer GPT forward pass (nn_GPT_21019569946962).

Sharding: sequence-parallel, 8 cores = 2 batches x 4 chunks of 256 tokens.
Per layer each core LayerNorms its 256 tokens, projects K (feature-major)
and V (token-major, via matmul with the activation as the stationary
operand — no PE transposes), and AllGathers K||V in one fused collective
within its 4-core replica group.  Attention runs with 128x128 PE-array
quadrant packing: head-pair scores are row-packed (two 64-deep
contractions concurrently), AV is col-packed (two 64-wide outputs
concurrently), softmax denominators accumulate via ones-matmuls into
packed PSUM rows.  Causal masking multiplies binary bf16 masks into the
exp'd scores on the DVE (all-SBUF 2-byte fast path).  The MLP and all
projections run at full 128x128 utilisation in bf16.  The LM head is
vocab-sharded within each 4-core group (12576 columns per core over its
batch's 1024 tokens); PSUM results DMA straight to DRAM.

Activations flow feature-major [D, tokens]; weights stream from HBM in
bf16; the residual stream and LN/softmax statistics stay fp32.
"""

import sys

sys.path.insert(0, "/opt/trn_rl_repo")

import numpy as np
import ml_dtypes

import concourse.bass as bass
import concourse.tile as tile
import concourse.mybir as mybir
from concourse import bacc
from concourse import bass_utils

BF16 = mybir.dt.bfloat16
F32 = mybir.dt.float32
AF = mybir.ActivationFunctionType
ALU = mybir.AluOpType

import os
SKIP_COLL = os.environ.get("SKIP_COLL", "0") == "1"

N_CORES = 8
NL = 6          # layers
D = 768
DT = 6          # d-tiles of 128
H = 12          # heads
HD = 64         # head dim
DFF = 3072
DFT = 24        # dff tiles of 128
VOC = 50304
VS = VOC // 4   # 12576 vocab shard per core (4-way within batch group)
B, L = 2, 1024
TOK = 256       # tokens per core
GTOK = 1024     # tokens per replica group (one batch)
EPS = 1e-6
NSC = 4 * DT + DFT + DT   # packed per-layer scales: ln1s,ln1b,ln2s,ln2b,w1b,w2b
VCH = 384       # lm-head vocab chunk
NVCH = 32       # full chunks; remainder 288
VREM = VS - NVCH * VCH


class GptKernel:
    def __init__(self, reps=1):
        self.reps = reps
        self.nc = self._build()

    # -------------------------------------------------------------- build
    def _build(self):
        nc = bacc.Bacc("TRN2", target_bir_lowering=False, debug=False,
                       enable_asserts=True, num_devices=N_CORES)
        self.nc = nc

        def din(name, shape, dt):
            return nc.dram_tensor(name, shape, dt, kind="ExternalInput").ap()

        self.x0 = din("x0", [D, TOK], F32)
        self.wq = din("wq", [NL, D, D], BF16)
        self.wk = din("wk", [NL, D, D], BF16)
        self.wv = din("wv", [NL, D, D], BF16)
        self.wo = din("wo", [NL, D, D], BF16)
        self.w1 = din("w1", [NL, D, DFF], BF16)
        self.w2 = din("w2", [NL, DFF, D], BF16)
        self.scal = din("scal", [NL, 128, NSC], F32)
        self.lnfs = din("lnfs", [D], F32)
        self.lnfb = din("lnfb", [D], F32)
        self.headw = din("headw", [D, VS], BF16)
        self.amask = din("amask", [8, 128, 2 * TOK], BF16)
        self.selc = din("selc", [2, 128, 128], BF16)
        self.out = nc.dram_tensor("out", [GTOK, VS], F32,
                                  kind="ExternalOutput").ap()

        with tile.TileContext(nc) as tc:
            self.tc = tc
            with (
                tc.tile_pool(name="const", bufs=1) as cp,
                tc.tile_pool(name="persist", bufs=1) as pp,
                tc.tile_pool(name="psum", bufs=1, space="PSUM") as psum,
                tc.tile_pool(name="dram", bufs=1, space="DRAM") as dram,
                tc.tile_pool(name="work", bufs=1) as wp,
            ):
                self.psum, self.dram, self.wp = psum, dram, wp
                self.ones_r = cp.tile([1, 128], F32)
                nc.vector.memset(self.ones_r[:], 1.0)
                self.ones_c = cp.tile([128, 1], BF16)
                nc.vector.memset(self.ones_c[:], 1.0)
                self.sel = cp.tile([128, 2, 128], BF16)
                nc.sync.dma_start(self.sel[:],
                                  self.selc.rearrange("s p q -> p s q"))
                self.mask_sb = pp.tile([128, 8, 2 * TOK], BF16)
                nc.sync.dma_start(self.mask_sb[:],
                                  self.amask.rearrange("k p t -> p k t"))
                self.xres = pp.tile([128, DT, TOK], F32)

                for rep in range(self.reps):
                    nc.sync.dma_start(
                        self.xres[:],
                        self.x0.rearrange("(dt p) t -> p dt t", p=128))
                    for l in range(NL):
                        self._layer(l, rep)
                    self._lm_head(rep)
        nc.compile()
        return nc

    # ------------------------------------------------------------ layernorm
    def _layernorm(self, xres, g, b, name):
        """xres [128, DT, TOK] f32 -> ln [128, DT, TOK] bf16."""
        nc, wp, psum = self.nc, self.wp, self.psum
        stat = psum.tile([128, TOK], F32, tag="mm", bufs=2, name=f"st_{name}")
        xbs = []
        for k in range(DT):
            xb = wp.tile([128, TOK], BF16, tag="xb", bufs=3, name=f"xb{k}_{name}")
            nc.vector.tensor_copy(xb[:], xres[:, k, :])
            xbs.append(xb)
        xqs = []
        for k in range(DT):
            xq = wp.tile([128, TOK], BF16, tag="xq", bufs=3, name=f"xq{k}_{name}")
            nc.scalar.activation(xq[:], xres[:, k, :], AF.Square,
                                 bias=0.0, scale=1.0)
            xqs.append(xq)
        for k in range(DT):
            nc.tensor.matmul(stat[0:1, :], self.ones_c[:], xbs[k][:],
                             start=(k == 0), stop=(k == DT - 1),
                             tile_position=(0, 0), skip_group_check=True)
        for k in range(DT):
            nc.tensor.matmul(stat[32:33, :], self.ones_c[:], xqs[k][:],
                             start=(k == 0), stop=(k == DT - 1),
                             tile_position=(0, 32), skip_group_check=True)
        mu = wp.tile([1, TOK], F32, tag="lnsc", bufs=8, name=f"mu_{name}")
        nc.vector.tensor_scalar_mul(mu[:], stat[0:1, :], 1.0 / D)
        msq = wp.tile([1, TOK], F32, tag="lnsc", bufs=8, name=f"msq_{name}")
        nc.vector.tensor_scalar_mul(msq[:], stat[32:33, :], 1.0 / D)
        mu2 = wp.tile([1, TOK], F32, tag="lnsc", bufs=8, name=f"mu2_{name}")
        nc.vector.tensor_mul(mu2[:], mu[:], mu[:])
        var = wp.tile([1, TOK], F32, tag="lnsc", bufs=8, name=f"va_{name}")
        nc.vector.tensor_sub(var[:], msq[:], mu2[:])
        vare = wp.tile([1, TOK], F32, tag="lnsc", bufs=8, name=f"ve_{name}")
        nc.vector.tensor_scalar_add(vare[:], var[:], EPS)
        sd = wp.tile([1, TOK], F32, tag="lnsc", bufs=8, name=f"sd_{name}")
        nc.scalar.activation(sd[:], vare[:], AF.Sqrt, bias=0.0, scale=1.0)
        rstd = wp.tile([1, TOK], F32, tag="lnsc", bufs=8, name=f"rstd_{name}")
        nc.vector.reciprocal(rstd[:], sd[:])
        nmr = wp.tile([1, TOK], F32, tag="lnsc", bufs=8, name=f"nmr_{name}")
        nc.vector.tensor_mul(nmr[:], mu[:], rstd[:])

        # bc[:, 0:TOK] = rstd broadcast, bc[:, TOK:2T] = mu*rstd broadcast
        bc = psum.tile([128, 2 * TOK], F32, tag="s", bufs=2, name=f"bc_{name}")
        nc.tensor.matmul(bc[:, 0:TOK], self.ones_r[:], rstd[:],
                         start=True, stop=True, skip_group_check=True)
        nc.tensor.matmul(bc[:, TOK:2 * TOK], self.ones_r[:], nmr[:],
                         start=True, stop=True, skip_group_check=True)

        ln = wp.tile([128, DT, TOK], BF16, tag=f"ln_{name[:3]}", bufs=1,
                     name=f"ln_{name}")
        for k in range(DT):
            u = wp.tile([128, TOK], F32, tag="lnu", bufs=2, name=f"u{k}_{name}")
            nc.vector.tensor_mul(u[:], xres[:, k, :], bc[:, 0:TOK])
            v = wp.tile([128, TOK], F32, tag="lnv", bufs=2, name=f"v{k}_{name}")
            nc.vector.tensor_sub(v[:], u[:], bc[:, TOK:2 * TOK])
            nc.scalar.activation(ln[:, k, :], v[:], AF.Identity,
                                 bias=b[:, k:k + 1], scale=g[:, k:k + 1])
        return ln

    # ------------------------------------------------------------ layer
    def _layer(self, l, rep):
        nc, wp, psum, dram = self.nc, self.wp, self.psum, self.dram
        nm = f"r{rep}l{l}"

        sc = wp.tile([128, NSC], F32, tag="sc", bufs=2, name=f"sc_{nm}")
        nc.sync.dma_start(sc[:], self.scal[l])
        g1, b1 = sc[:, 0:DT], sc[:, DT:2 * DT]
        g2, b2 = sc[:, 2 * DT:3 * DT], sc[:, 3 * DT:4 * DT]
        w1b = sc[:, 4 * DT:4 * DT + DFT]
        w2b = sc[:, 4 * DT + DFT:NSC]

        wq_sb = wp.tile([128, DT, D], BF16, tag="wq", bufs=1, name=f"wq_{nm}")
        nc.sync.dma_start(wq_sb[:],
                          self.wq[l].rearrange("(t p) d -> p t d", p=128))
        wk_sb = wp.tile([128, DT, D], BF16, tag="wk", bufs=1, name=f"wk_{nm}")
        nc.sync.dma_start(wk_sb[:],
                          self.wk[l].rearrange("(t p) d -> p t d", p=128))
        wv_sb = wp.tile([128, DT, D], BF16, tag="wv", bufs=1, name=f"wv_{nm}")
        nc.sync.dma_start(wv_sb[:],
                          self.wv[l].rearrange("(t p) d -> p t d", p=128))
        wo_sb = wp.tile([128, DT, D], BF16, tag="wo", bufs=1, name=f"wo_{nm}")
        nc.sync.dma_start(wo_sb[:],
                          self.wo[l].rearrange("(t p) d -> p t d", p=128))

        ln1 = self._layernorm(self.xres, g1, b1, f"ln1_{nm}")

        # ---- K projection (feature-major) + V projection (token-major)
        # into one staging tile, then a single fused AllGather.
        kvst = wp.tile([128, DT * TOK + 2 * D], BF16, tag="kvst", bufs=1,
                       name=f"kvst_{nm}")
        for m in range(DT):
            ps = psum.tile([128, TOK], F32, tag="mm", bufs=2,
                           name=f"pk{m}_{nm}")
            for kk in range(DT):
                nc.tensor.matmul(ps[:], wk_sb[:, kk, m * 128:(m + 1) * 128],
                                 ln1[:, kk, :],
                                 start=(kk == 0), stop=(kk == DT - 1))
            nc.vector.tensor_copy(kvst[:, m * TOK:(m + 1) * TOK], ps[:])
        for tb in range(2):
            for vh in range(2):
                ps = psum.tile([128, D // 2], F32, tag="mm", bufs=2,
                               name=f"pv{tb}_{vh}_{nm}")
                for kk in range(DT):
                    nc.tensor.matmul(ps[:],
                                     ln1[:, kk, tb * 128:(tb + 1) * 128],
                                     wv_sb[:, kk, vh * 384:(vh + 1) * 384],
                                     start=(kk == 0), stop=(kk == DT - 1),
                                     skip_group_check=True)
                off = DT * TOK + tb * D + vh * 384
                nc.vector.tensor_copy(kvst[:, off:off + 384], ps[:])

        kvin = dram.tile([128, DT * TOK + 2 * D], BF16, tag="kvin", bufs=2,
                         name=f"kvin_{nm}")
        nc.sync.dma_start(kvin[:], kvst[:])
        kvout = dram.tile([4, 128, DT * TOK + 2 * D], BF16, tag="kvout",
                          bufs=2, name=f"kvout_{nm}")
        if not SKIP_COLL:
            nc.gpsimd.collective_compute(
                "AllGather", ALU.bypass, ins=[kvin.opt()], outs=[kvout.opt()],
                replica_groups=[[0, 1, 2, 3], [4, 5, 6, 7]])

        # ---- Q projection (overlaps the gather)
        q_sb = wp.tile([128, DT, TOK], BF16, tag="q", bufs=1, name=f"q_{nm}")
        for m in range(DT):
            ps = psum.tile([128, TOK], F32, tag="mm", bufs=2,
                           name=f"pq{m}_{nm}")
            for kk in range(DT):
                nc.tensor.matmul(ps[:], wq_sb[:, kk, m * 128:(m + 1) * 128],
                                 ln1[:, kk, :],
                                 start=(kk == 0), stop=(kk == DT - 1))
            nc.vector.tensor_copy(q_sb[:, m, :], ps[:])
        q64 = wp.tile([64, 2, DT, TOK], BF16, tag="q64", bufs=1,
                      name=f"q64_{nm}")
        for h2 in range(2):
            nc.sync.dma_start(q64[:, h2], q_sb[64 * h2:64 * h2 + 64])

        # ---- load gathered K (feature-major) and V^T (token-major)
        kgr, vtr = [], []
        for r in range(4):
            kt = wp.tile([64, 2, DT, TOK], BF16, tag="kg", bufs=4,
                         name=f"kg{r}_{nm}")
            for h2 in range(2):
                nc.sync.dma_start(
                    kt[:, h2],
                    kvout[r, 64 * h2:64 * h2 + 64, 0:DT * TOK].rearrange(
                        "p (dt t) -> p dt t", dt=DT))
            kgr.append(kt)
            vv = wp.tile([128, 2, D], BF16, tag="vt", bufs=4, name=f"vt{r}_{nm}")
            nc.sync.dma_start(
                vv[:], kvout[r, :, DT * TOK:].rearrange("p (tb d) -> p tb d",
                                                        tb=2))
            vtr.append(vv)

        # ---- attention: head pairs, quadrant-packed
        at = wp.tile([128, DT, TOK], BF16, tag="at", bufs=1, name=f"at_{nm}")
        for j in range(DT):
            ao = psum.tile([128, TOK], F32, tag="ao", bufs=2, name=f"ao{j}_{nm}")
            dn = psum.tile([128, TOK], F32, tag="dn", bufs=2,
                           name=f"dn{j}_{nm}")
            for kb in range(8):
                s = psum.tile([128, 2 * TOK], F32, tag="s", bufs=2,
                              name=f"s{j}_{kb}_{nm}")
                r, tb = kb // 2, kb % 2
                nc.tensor.matmul(
                    s[:, 0:TOK], kgr[r][:, 0, j, tb * 128:(tb + 1) * 128],
                    q64[:, 0, j, :], start=True, stop=True)
                nc.tensor.matmul(
                    s[:, TOK:2 * TOK], kgr[r][:, 1, j, tb * 128:(tb + 1) * 128],
                    q64[:, 1, j, :], start=True, stop=True)
                pm = wp.tile([128, 2 * TOK], BF16, tag="pm", bufs=3,
                             name=f"pm{j}_{kb}_{nm}")
                nc.scalar.activation(pm[:], s[:], AF.Exp, bias=0.0, scale=0.125)
                p = wp.tile([128, 2 * TOK], BF16, tag="p", bufs=3,
                            name=f"p{j}_{kb}_{nm}")
                nc.vector.tensor_mul(p[:], pm[:], self.mask_sb[:, kb, :])
                nc.tensor.matmul(ao[0:64, :],
                                 vtr[r][:, tb, 2 * j * 64:2 * j * 64 + 64],
                                 p[:, 0:TOK], start=(kb == 0), stop=(kb == 7),
                                 skip_group_check=True)
                nc.tensor.matmul(ao[64:128, :],
                                 vtr[r][:, tb, (2 * j + 1) * 64:(2 * j + 2) * 64],
                                 p[:, TOK:2 * TOK], start=(kb == 0), stop=(kb == 7),
                                 skip_group_check=True)
                nc.tensor.matmul(dn[0:1, :], self.ones_c[:],
                                 p[:, 0:TOK], start=(kb == 0), stop=(kb == 7),
                                 tile_position=(0, 0), skip_group_check=True)
                nc.tensor.matmul(dn[64:65, :], self.ones_c[:],
                                 p[:, TOK:2 * TOK], start=(kb == 0), stop=(kb == 7),
                                 tile_position=(0, 64), skip_group_check=True)
            rd = wp.tile([128, TOK], BF16, tag="rd", bufs=2,
                         name=f"rd{j}_{nm}")
            nc.vector.memset(rd[:], 0.0)
            with nc.allow_low_precision(reason="softmax denom bcast in bf16"):
                nc.vector.reciprocal(rd[0:1, :], dn[0:1, :])
                nc.vector.reciprocal(rd[64:65, :], dn[64:65, :])
            bc = psum.tile([128, TOK], F32, tag="s", bufs=2,
                           name=f"bc{j}_{nm}")
            nc.tensor.matmul(bc[:], self.sel[:, 0, :], rd[:],
                             start=True, stop=True, skip_group_check=True)
            aosb = wp.tile([128, TOK], BF16, tag="aosb", bufs=2,
                           name=f"aosb{j}_{nm}")
            nc.vector.tensor_copy(aosb[:], ao[:])
            nc.vector.tensor_mul(at[:, j, :], aosb[:], bc[:])

        # ---- WO + residual
        for m in range(DT):
            ps = psum.tile([128, TOK], F32, tag="mm", bufs=2,
                           name=f"pwo{m}_{nm}")
            for j in range(DT):
                nc.tensor.matmul(ps[:], wo_sb[:, j, m * 128:(m + 1) * 128],
                                 at[:, j, :], start=(j == 0), stop=(j == DT - 1))
            nc.vector.tensor_add(self.xres[:, m, :], self.xres[:, m, :], ps[:])

        # ---- LN2 + MLP
        ln2 = self._layernorm(self.xres, g2, b2, f"ln2_{nm}")
        h1 = wp.tile([128, DFT, TOK], BF16, tag="h1", bufs=1, name=f"h1_{nm}")
        for blk in range(4):
            w1_sb = wp.tile([128, DT, DFF // 4], BF16, tag="w1", bufs=2,
                            name=f"w1_{blk}_{nm}")
            nc.sync.dma_start(
                w1_sb[:],
                self.w1[l, :, blk * 768:(blk + 1) * 768].rearrange(
                    "(t p) d -> p t d", p=128))
            for mi in range(6):
                m = blk * 6 + mi
                ps = psum.tile([128, TOK], F32, tag="mm", bufs=2,
                               name=f"ph1_{m}_{nm}")
                for kk in range(DT):
                    nc.tensor.matmul(ps[:],
                                     w1_sb[:, kk, mi * 128:(mi + 1) * 128],
                                     ln2[:, kk, :],
                                     start=(kk == 0), stop=(kk == DT - 1))
                nc.scalar.activation(h1[:, m, :], ps[:], AF.Gelu_apprx_tanh,
                                     bias=w1b[:, m:m + 1], scale=1.0)
        for blk in range(3):
            w2_sb = wp.tile([128, DFT, 2 * 128], BF16, tag="w2", bufs=2,
                            name=f"w2_{blk}_{nm}")
            nc.sync.dma_start(
                w2_sb[:],
                self.w2[l, :, blk * 256:(blk + 1) * 256].rearrange(
                    "(t p) d -> p t d", p=128))
            for mi in range(2):
                m = blk * 2 + mi
                ps = psum.tile([128, TOK], F32, tag="mm", bufs=2,
                               name=f"pw2_{m}_{nm}")
                for kk in range(DFT):
                    nc.tensor.matmul(ps[:],
                                     w2_sb[:, kk, mi * 128:(mi + 1) * 128],
                                     h1[:, kk, :],
                                     start=(kk == 0), stop=(kk == DFT - 1))
                mo = wp.tile([128, TOK], F32, tag="mo", bufs=2, name=f"mo{m}_{nm}")
                nc.scalar.activation(mo[:], ps[:], AF.Identity,
                                     bias=w2b[:, m:m + 1], scale=1.0)
                nc.vector.tensor_add(self.xres[:, m, :], self.xres[:, m, :], mo[:])

    # ------------------------------------------------------------ lm head
    def _lm_head(self, rep):
        nc, wp, psum, dram = self.nc, self.wp, self.psum, self.dram
        nm = f"r{rep}f"
        gf = wp.tile([128, DT], F32, tag="sc", bufs=2, name=f"gf_{nm}")
        nc.sync.dma_start(gf[:], self.lnfs.rearrange("(t p) -> p t", p=128))
        bf = wp.tile([128, DT], F32, tag="sc", bufs=2, name=f"bf_{nm}")
        nc.sync.dma_start(bf[:], self.lnfb.rearrange("(t p) -> p t", p=128))
        lnf = self._layernorm(self.xres, gf, bf, f"lnf_{nm}")

        fin = dram.tile([128, DT * TOK], BF16, tag="kvin", bufs=2,
                        name=f"fin_{nm}")
        nc.sync.dma_start(fin[:], lnf[:].rearrange("p t d -> p (t d)"))
        fout = dram.tile([4, 128, DT * TOK], BF16, tag="kvout", bufs=2,
                         name=f"fout_{nm}")
        if not SKIP_COLL:
            nc.gpsimd.collective_compute(
                "AllGather", ALU.bypass, ins=[fin.opt()], outs=[fout.opt()],
                replica_groups=[[0, 1, 2, 3], [4, 5, 6, 7]])
        fgr = []
        for r in range(4):
            ft = wp.tile([128, DT, TOK], BF16, tag="kg", bufs=4,
                         name=f"fg{r}_{nm}")
            nc.sync.dma_start(ft[:], fout[r].rearrange("p (dt t) -> p dt t",
                                                       dt=DT))
            fgr.append(ft)

        chunks = [(c * VCH, VCH) for c in range(NVCH)] + [(NVCH * VCH, VREM)]
        for c0, cn in chunks:
            hw = wp.tile([128, DT, VCH], BF16, tag="hw", bufs=2,
                         name=f"hw{c0}_{nm}")
            nc.sync.dma_start(
                hw[:, :, 0:cn],
                self.headw[:, c0:c0 + cn].rearrange("(t p) v -> p t v", p=128))
            for tb in range(8):
                ot = wp.tile([128, VCH], F32, tag="ot", bufs=2,
                             name=f"ot{c0}_{tb}_{nm}")
                for si, s0 in enumerate(range(0, cn, 512)):
                    sn = min(512, cn - s0)
                    ps = psum.tile([128, 512], F32, tag="s", bufs=2,
                                   name=f"hp{c0}_{tb}_{si}_{nm}")
                    for kk in range(DT):
                        nc.tensor.matmul(
                            ps[:, 0:sn],
                            fgr[tb // 2][:, kk, (tb % 2) * 128:(tb % 2 + 1) * 128],
                            hw[:, kk, s0:s0 + sn],
                            start=(kk == 0), stop=(kk == DT - 1),
                            skip_group_check=True)
                    if si % 2 == 0:
                        nc.vector.tensor_copy(ot[:, s0:s0 + sn], ps[:, 0:sn])
                    else:
                        nc.scalar.activation(ot[:, s0:s0 + sn], ps[:, 0:sn],
                                             AF.Identity, bias=0.0, scale=1.0)
                nc.sync.dma_start(
                    self.out[tb * 128:(tb + 1) * 128, c0:c0 + cn],
                    ot[:, 0:cn])


# ------------------------------------------------------------------ host side

_CACHE = {}


def _prep_inputs(inputs):
    ids = np.asarray(inputs["input_ids"])
    tok_emb = np.asarray(inputs["tok_emb"], dtype=np.float32)
    pos_emb = np.asarray(inputs["pos_emb"], dtype=np.float32)
    x = tok_emb[ids] + pos_emb[:L][None]          # [2, 1024, 768] f32

    bf = lambda a: np.ascontiguousarray(np.asarray(a, np.float32)).astype(ml_dtypes.bfloat16)
    f32 = lambda a: np.ascontiguousarray(np.asarray(a, np.float32))

    # packed per-layer scales: [NL, 128, NSC]; column k of row p is element
    # (k*128+p) of the flat [768] / [3072] vectors (partition-major tiles)
    scal = np.zeros((NL, 128, NSC), np.float32)
    def pack(dst_off, src, width):
        # src [NL, width*128] -> scal[:, :, dst_off:dst_off+width]
        scal[:, :, dst_off:dst_off + width] = src.reshape(NL, width, 128).transpose(0, 2, 1)
    pack(0, f32(inputs["ln1_s"]), DT)
    pack(DT, f32(inputs["ln1_b"]), DT)
    pack(2 * DT, f32(inputs["ln2_s"]), DT)
    pack(3 * DT, f32(inputs["ln2_b"]), DT)
    pack(4 * DT, f32(inputs["w1_b"]), DFT)
    pack(4 * DT + DFT, f32(inputs["w2_b"]), DT)

    shared = {
        "wq": bf(inputs["wq"]), "wk": bf(inputs["wk"]),
        "wv": bf(inputs["wv"]), "wo": bf(inputs["wo"]),
        "w1": bf(inputs["w1_k"]), "w2": bf(inputs["w2_k"]),
        "scal": scal,
        "lnfs": f32(inputs["lnf_s"]), "lnfb": f32(inputs["lnf_b"]),
    }
    head_bf = bf(inputs["head"])

    # selector constants for denominator broadcast: pattern s (0: rows 0/32,
    # 1: rows 64/96): sel[s][k, p] = 1 iff k == 64*s + 32*(p >= 64)
    selc = np.zeros((2, 128, 128), ml_dtypes.bfloat16)
    selc[0, 0, 0:64] = 1.0
    selc[0, 64, 64:128] = 1.0

    in_maps = []
    for c in range(N_CORES):
        g, j = c // 4, c % 4
        m = dict(shared)
        m["x0"] = np.ascontiguousarray(x[g, TOK * j:TOK * (j + 1)].T)
        m["headw"] = np.ascontiguousarray(head_bf[:, j * VS:(j + 1) * VS])
        m["selc"] = selc
        # binary causal mask, duplicated for the head pair: [8, 128, 512]
        am = np.zeros((8, 128, 2 * TOK), ml_dtypes.bfloat16)
        for kb in range(8):
            kgl = 128 * kb + np.arange(128)[:, None]
            qgl = TOK * j + np.arange(TOK)[None, :]
            vis = (kgl <= qgl).astype(ml_dtypes.bfloat16)
            am[kb, :, 0:TOK] = vis
            am[kb, :, TOK:2 * TOK] = vis
        m["amask"] = am
        in_maps.append(m)
    return in_maps


def _assemble(results):
    final = np.empty((B, L, VOC), np.float32)
    for c in range(N_CORES):
        g, j = c // 4, c % 4
        final[g, :, j * VS:(j + 1) * VS] = results[c]["out"]
    return final


def kernel(**inputs):
    if "k" not in _CACHE:
        _CACHE["k"] = GptKernel(reps=1)
    gk = _CACHE["k"]
    in_maps = _prep_inputs(inputs)
    res = bass_utils.run_bass_kernel_spmd(
        gk.nc, in_maps, core_ids=list(range(N_CORES)))
    _CACHE["last_results"] = res
    return _assemble(res.results)
